# revision 1
# baseline (speedup 1.0000x reference)
"""Multi-head attention (B=2, N=M=2048, D=1024, H=16) on 8 Trainium2 cores.

Sharding: data-parallel over batch (cores 0-3 -> batch 0, cores 4-7 -> batch 1),
tensor-parallel over heads (4 heads per core). Each core computes

    qT  = (Wq_shard @ query_b.T + bq_shard)          # [256, 2048]  (head-dim major)
    kT  = (Wk_shard @ key_b.T   + bk_shard)          # [256, 2048]
    v   = (value_b @ Wv_shard.T + bv_shard)          # [2048, 256]  (key major)
    per head h (4 local heads):
        sT   = kT_h.T-contraction:  sT[key, q] = sum_hd kT[hd,key] qT[hd,q]
        eT   = exp(0.125 * sT)                       # [2048key, 2048q]
        o/den= [v_h | 1].T @ eT                      # [65, q] rows 0-63 out, row 64 denom
        outT_h = o * (1/den)                         # [64, q]
    partial = outT.T @ WpT_shard                     # [2048, 1024]

Host sums the 4 per-batch partials and adds bp.

All matmul inputs are bf16 (fp32 accumulation in PSUM); softmax statistics in
fp32. No row-max subtraction: scores are ~N(0,1) so exp() cannot overflow.
"""

import os

import numpy as np
import ml_dtypes

B, N, M, D, H = 2, 2048, 2048, 1024, 16
HD = D // H            # 64
NCORES = 8
HPC = 4                # heads per core
SH = HPC * HD          # 256, projected dim shard per core
P = 128
CH = 512               # query-chunk (matmul moving free dim)
NCH = N // CH          # 4
KT = M // P            # 16 key tiles
ET = D // P            # 8 embedding k-tiles
KO = SH // P           # 2 head-dim k-tiles ("m tiles")
SCALE = float(HD) ** -0.5

_CACHED_NC = None
LAST_RESULT = None     # BassKernelResults of the most recent run (for test harness)


def _build_bass():
    import concourse.bass as bass
    import concourse.mybir as mybir
    import concourse.tile as tile
    from concourse import bacc
    from concourse.bass import ts

    BF = mybir.dt.bfloat16
    F32 = mybir.dt.float32
    EXP = mybir.ActivationFunctionType.Exp

    nc = bacc.Bacc()

    qT_d = nc.dram_tensor("qT_in", [D, N], BF, kind="ExternalInput")
    kT_d = nc.dram_tensor("kT_in", [D, M], BF, kind="ExternalInput")
    vT_d = nc.dram_tensor("vT_in", [D, M], BF, kind="ExternalInput")
    wqT_d = nc.dram_tensor("wqT", [D, SH], BF, kind="ExternalInput")
    wkT_d = nc.dram_tensor("wkT", [D, SH], BF, kind="ExternalInput")
    wvT_d = nc.dram_tensor("wvT", [D, SH], BF, kind="ExternalInput")
    wpT_d = nc.dram_tensor("wpT", [SH, D], BF, kind="ExternalInput")
    bq_d = nc.dram_tensor("bq2", [KO, P], F32, kind="ExternalInput")
    bk_d = nc.dram_tensor("bk2", [KO, P], F32, kind="ExternalInput")
    bvb_d = nc.dram_tensor("bvb", [P, SH], F32, kind="ExternalInput")
    out_d = nc.dram_tensor("out_partial", [N, D], F32, kind="ExternalOutput")

    with tile.TileContext(nc) as tc:
        with (
            tc.tile_pool(name="consts", bufs=1) as consts,
            tc.tile_pool(name="stage", bufs=8) as stage,
            tc.tile_pool(name="acts", bufs=1) as acts,
            tc.tile_pool(name="exp_pool", bufs=2) as exp_pool,
            tc.tile_pool(name="misc", bufs=3) as misc,
            tc.tile_pool(name="outsb", bufs=3) as outsb,
            tc.tile_pool(name="dram_scratch", bufs=3, space="DRAM") as dram_scratch,
            tc.tile_pool(name="mm_ps", bufs=2, space="PSUM") as mm_ps,
            tc.tile_pool(name="scores_ps", bufs=2, space="PSUM") as scores_ps,
            tc.tile_pool(name="out_ps", bufs=2, space="PSUM") as out_ps,
        ):
            wp_sb = [None]  # loaded in v_proj (late DMA)

            # ---- persistent activations ----
            qT_sb = acts.tile([P, KO, N], BF, name="qT_sb")     # [hd128, mtile, q]
            kT_sb = acts.tile([P, KO, M], BF, name="kT_sb")     # [hd128, mtile, key]
            # v plus a ones column per head: [key128, ktile, head, 65]
            vaug_sb = acts.tile([P, KT, HPC, HD + 1], BF, name="vaug_sb")
            outT_sb = acts.tile([P, KO, N], BF, name="outT_sb")  # normalized attn out.T

            nc.vector.memset(vaug_sb[:, :, :, HD : HD + 1], 1.0)

            # ---- phase A: projections ----
            # qin and kin get separate slot tags so key tiles prefetch while
            # the Q projection runs; vin reuses qin's slots afterwards.
            def load_tiles(dram, tag, slot_tag):
                tls = []
                for k in range(ET):
                    t = stage.tile([P, N], BF, name=f"{tag}{k}", tag=slot_tag)
                    nc.sync.dma_start(out=t, in_=dram[ts(k, P), :])
                    tls.append(t)
                return tls

            # weights first (small), then the 8 MB of q/k input tiles that
            # gate the first score matmuls — measured fastest order
            wq_sb = consts.tile([P, ET, SH], BF, name="wq_sb")
            nc.sync.dma_start(out=wq_sb, in_=wqT_d.rearrange("(ko p) m -> p ko m", p=P))
            wk_sb = consts.tile([P, ET, SH], BF, name="wk_sb")
            nc.sync.dma_start(out=wk_sb, in_=wkT_d.rearrange("(ko p) m -> p ko m", p=P))
            bq_sb = consts.tile([P, KO], F32, name="bq_sb")
            nc.sync.dma_start(out=bq_sb, in_=bq_d.rearrange("t p -> p t"))
            bk_sb = consts.tile([P, KO], F32, name="bk_sb")
            nc.sync.dma_start(out=bk_sb, in_=bk_d.rearrange("t p -> p t"))
            qin = load_tiles(qT_d, "qin", "stage_qv")
            kin = load_tiles(kT_d, "kin", "stage_k")

            def qk_proj(m):
                """q and k projections for head-pair (m-tile) m (k-inner)."""
                for c in range(NCH):
                    ps = mm_ps.tile([P, CH], F32, name="ps_q", tag="mm")
                    for k in range(ET):
                        nc.tensor.matmul(
                            ps, lhsT=wq_sb[:, k, ts(m, P)], rhs=qin[k][:, ts(c, CH)],
                            start=(k == 0), stop=(k == ET - 1),
                        )
                    nc.vector.tensor_scalar_add(
                        out=qT_sb[:, m, ts(c, CH)], in0=ps, scalar1=bq_sb[:, m : m + 1]
                    )
                for c in range(NCH):
                    ps = mm_ps.tile([P, CH], F32, name="ps_k", tag="mm")
                    for k in range(ET):
                        nc.tensor.matmul(
                            ps, lhsT=wk_sb[:, k, ts(m, P)], rhs=kin[k][:, ts(c, CH)],
                            start=(k == 0), stop=(k == ET - 1),
                        )
                    nc.vector.tensor_scalar_add(
                        out=kT_sb[:, m, ts(c, CH)], in0=ps, scalar1=bk_sb[:, m : m + 1]
                    )

            def qk_proj_fast(m):
                """k-outer q/k projections for head-pair m: psums for all of
                kT (scores pool, idle in phase A) plus the first two q chunks
                (mm pool) are live at once, so every input tile is consumed
                the moment its DMA lands. scores(c=0, hp=m) can start right
                after the last input tile arrives."""
                kpss = [
                    scores_ps.tile([P, 2, CH], F32, name=f"kp{i}", tag="sc")
                    for i in range(2)
                ]
                # q chunks 0-1 on the mm pool, chunks 2-3 on the out pool
                # (both idle in phase A): all 8 q/k chunk psums live at once
                qps = [
                    mm_ps.tile([P, CH], F32, name=f"qp{i}", tag="mm")
                    for i in range(2)
                ] + [
                    out_ps.tile([P, CH], F32, name=f"qo{i}", tag="ops")
                    for i in range(2)
                ]
                for k in range(ET):
                    for c in range(NCH):
                        nc.tensor.matmul(
                            kpss[c // 2][:, c % 2, :],
                            lhsT=wk_sb[:, k, ts(m, P)],
                            rhs=kin[k][:, ts(c, CH)],
                            start=(k == 0), stop=(k == ET - 1),
                        )
                    for c in range(NCH):
                        nc.tensor.matmul(
                            qps[c],
                            lhsT=wq_sb[:, k, ts(m, P)],
                            rhs=qin[k][:, ts(c, CH)],
                            start=(k == 0), stop=(k == ET - 1),
                        )
                for c in range(NCH):
                    nc.vector.tensor_scalar_add(
                        out=kT_sb[:, m, ts(c, CH)],
                        in0=kpss[c // 2][:, c % 2, :],
                        scalar1=bk_sb[:, m : m + 1],
                    )
                for c in range(NCH):
                    nc.vector.tensor_scalar_add(
                        out=qT_sb[:, m, ts(c, CH)],
                        in0=qps[c],
                        scalar1=bq_sb[:, m : m + 1],
                    )
            def q_tail(m):
                """q projection chunks 2..3 for head-pair m."""
                qps2 = [
                    mm_ps.tile([P, CH], F32, name=f"qq{i}", tag="mm")
                    for i in range(2)
                ]
                for k in range(ET):
                    for c in range(2, NCH):
                        nc.tensor.matmul(
                            qps2[c - 2],
                            lhsT=wq_sb[:, k, ts(m, P)],
                            rhs=qin[k][:, ts(c, CH)],
                            start=(k == 0), stop=(k == ET - 1),
                        )
                for c in range(2, NCH):
                    nc.vector.tensor_scalar_add(
                        out=qT_sb[:, m, ts(c, CH)],
                        in0=qps2[c - 2],
                        scalar1=bq_sb[:, m : m + 1],
                    )

            def v_proj():
                # wv/bvb/wp loads deferred to here: keeps the startup DMA
                # window free for the q/k inputs that gate the first exp
                wv_sb = consts.tile([P, ET, SH], BF, name="wv_sb")
                nc.sync.dma_start(
                    out=wv_sb, in_=wvT_d.rearrange("(ko p) m -> p ko m", p=P)
                )
                bvb_sb = consts.tile([P, SH], F32, name="bvb_sb")
                nc.sync.dma_start(out=bvb_sb, in_=bvb_d[:, :])
                wp_sb[0] = consts.tile([P, KO, D], BF, name="wp_sb")
                nc.sync.dma_start(
                    out=wp_sb[0], in_=wpT_d.rearrange("(ko p) n -> p ko n", p=P)
                )
                vin = load_tiles(vT_d, "vin", "stage_qv")
                for kt in range(KT):
                    ps = mm_ps.tile([P, CH], F32, name="ps_v", tag="mm")
                    for k in range(ET):
                        nc.tensor.matmul(
                            ps[:, :SH], lhsT=vin[k][:, ts(kt, P)], rhs=wv_sb[:, k, :],
                            start=(k == 0), stop=(k == ET - 1),
                        )
                    nc.vector.tensor_tensor(
                        out=vaug_sb[:, kt, :, 0:HD],
                        in0=ps[:, :SH].rearrange("p (h x) -> p h x", h=HPC),
                        in1=bvb_sb.rearrange("p (h x) -> p h x", h=HPC),
                        op=mybir.AluOpType.add,
                    )

            def scores_exp(c, hp):
                """scores + exp for (chunk, head-pair) -> expT tile.

                One 2-bank psum group per key-tile (both heads), double
                buffered, so each [128,1024] ACT exp overlaps the next
                key-tile's score matmuls."""
                expT = exp_pool.tile([P, 2 * KT, CH], BF, name="expT", tag="expT")
                for kt in range(KT):
                    sc = scores_ps.tile([P, 2, CH], F32, name="sc", tag="sc")
                    for ha in range(2):
                        pb = ha * HD
                        nc.tensor.matmul(
                            sc[:, ha, :],
                            lhsT=kT_sb[pb : pb + HD, hp, ts(kt, P)],
                            rhs=qT_sb[pb : pb + HD, hp, ts(c, CH)],
                            start=True, stop=True,
                        )
                    nc.scalar.activation(
                        out=expT[:, 2 * kt : 2 * kt + 2, :], in_=sc,
                        func=EXP, scale=SCALE,
                    )
                return expT

            def out_block(c, hp, expT):
                """attn @ [v|1], then normalize into outT_sb."""
                for ha in range(2):
                    hl = hp * 2 + ha
                    ops = out_ps.tile([P, CH], F32, name="ops", tag="ops")
                    for kt in range(KT):
                        j = 2 * kt + ha
                        nc.tensor.matmul(
                            ops[: HD + 1, :],
                            lhsT=vaug_sb[:, kt, hl, :],
                            rhs=expT[:, j, :],
                            start=(kt == 0), stop=(kt == KT - 1),
                        )
                    recip = misc.tile([1, CH], F32, name="recip", tag="recip")
                    nc.vector.reciprocal(recip, ops[HD : HD + 1, :])
                    rd = dram_scratch.tile([1, CH], F32, name="rd", tag="rd")
                    nc.sync.dma_start(out=rd, in_=recip)
                    rb = misc.tile([HD, CH], F32, name="rb", tag="rb")
                    nc.sync.dma_start(
                        out=rb,
                        in_=bass.AP(
                            tensor=rd.tensor,
                            offset=rd.offset,
                            ap=[[0, HD]] + rd.ap[1:],
                        ),
                    )
                    nc.vector.tensor_mul(
                        outT_sb[ha * HD : (ha + 1) * HD, hp, ts(c, CH)],
                        ops[:HD, :],
                        rb,
                    )

            def final_proj(c):
                for qt in range(c * CH // P, (c + 1) * CH // P):
                    for dc in range(D // CH):
                        fp = mm_ps.tile([P, CH], F32, name="fp", tag="mm")
                        for k2 in range(KO):
                            nc.tensor.matmul(
                                fp, lhsT=outT_sb[:, k2, ts(qt, P)],
                                rhs=wp_sb[0][:, k2, ts(dc, CH)],
                                start=(k2 == 0), stop=(k2 == KO - 1),
                            )
                        ob = outsb.tile([P, CH], F32, name="ob", tag="ob")
                        if c == NCH - 1:
                            # last chunk is the kernel tail: ACT is idle after
                            # its final exp, so evict there instead of DVE
                            nc.scalar.copy(ob, fp)
                        else:
                            nc.vector.tensor_copy(ob, fp)
                        nc.sync.dma_start(
                            out=out_d[ts(qt, P), ts(dc, CH)], in_=ob
                        )

            # Emission = per-engine program order. Software-pipelined: scores
            # of block n+1 are emitted between the out/normalize halves of
            # block n so ACT (the bottleneck) never waits on PE. expT stays
            # within 2 live buffers.
            qk_proj_fast(0)
            eT = {}
            eT[0, 0] = scores_exp(0, 0)
            qk_proj(1)
            eT[0, 1] = scores_exp(0, 1)
            v_proj()
            blocks = [(c, hp) for c in range(NCH) for hp in range(KO)]
            for i, (c, hp) in enumerate(blocks):
                out_block(c, hp, eT.pop((c, hp)))
                nxt = i + 2  # scores for block i+2 were not emitted yet
                if nxt < len(blocks):
                    eT[blocks[nxt]] = scores_exp(*blocks[nxt])
                if hp == KO - 1:
                    final_proj(c)
    nc.compile()
    return nc


def _get_nc():
    global _CACHED_NC
    if _CACHED_NC is None:
        _CACHED_NC = _build_bass()
    return _CACHED_NC


def _prep_in_maps(query, key, value, Wq, bq, Wk, bk, Wv, bv, Wp, bp):
    bf16 = ml_dtypes.bfloat16
    f32 = np.float32
    in_maps = []
    for core in range(NCORES):
        b = core // (NCORES // B)
        hs = (core % (NCORES // B)) * HPC * HD   # first head-dim of shard
        sl = slice(hs, hs + SH)
        m = {
            "qT_in": np.ascontiguousarray(query[b].T).astype(bf16),
            "kT_in": np.ascontiguousarray(key[b].T).astype(bf16),
            "vT_in": np.ascontiguousarray(value[b].T).astype(bf16),
            "wqT": np.ascontiguousarray(Wq[sl, :].T).astype(bf16),
            "wkT": np.ascontiguousarray(Wk[sl, :].T).astype(bf16),
            "wvT": np.ascontiguousarray(Wv[sl, :].T).astype(bf16),
            "wpT": np.ascontiguousarray(Wp[:, sl].T).astype(bf16),
            "bq2": np.ascontiguousarray(bq[sl]).astype(f32).reshape(KO, P),
            "bk2": np.ascontiguousarray(bk[sl]).astype(f32).reshape(KO, P),
            "bvb": np.tile(np.asarray(bv[sl], f32).reshape(1, SH), (P, 1)),
        }
        in_maps.append(m)
    return in_maps


class _Runner:
    """Reusable SPMD PJRT executor for a Bass module (axon or native PJRT).

    Mirrors bass2jax.run_bass_via_pjrt but keeps the jitted function so
    repeated (timed) executions don't rebuild/re-trace, and skips donation so
    input device buffers can be reused across calls (our kernel writes every
    output element, so pre-zeroed outputs are not required)."""

    def __init__(self, nc):
        import jax
        import concourse.mybir as mybir
        from concourse import bass2jax
        from jax.experimental.shard_map import shard_map
        from jax.sharding import Mesh, PartitionSpec

        bass2jax.install_neuronx_cc_hook()
        self.nc = nc
        self.jax = jax
        partition_name = (
            nc.partition_id_tensor.name if nc.partition_id_tensor else None
        )
        in_names, out_names, out_avals, zero_outs = [], [], [], []
        for alloc in nc.m.functions[0].allocations:
            if not isinstance(alloc, mybir.MemoryLocationSet):
                continue
            name = alloc.memorylocations[0].name
            if alloc.kind == "ExternalInput":
                if name != partition_name:
                    in_names.append(name)
            elif alloc.kind == "ExternalOutput":
                shape = tuple(alloc.tensor_shape)
                dtype = mybir.dt.np(alloc.dtype)
                out_names.append(name)
                out_avals.append(jax.core.ShapedArray(shape, dtype))
                zero_outs.append(np.zeros(shape, dtype))
        self.in_names = list(in_names)
        self.out_names = out_names
        self.out_avals = out_avals
        self.zero_outs = zero_outs
        n_params = len(in_names)
        all_in_names = in_names + out_names
        if partition_name is not None:
            all_in_names.append(partition_name)

        def _body(*args):
            operands = list(args)
            if partition_name is not None:
                operands.append(bass2jax.partition_id_tensor())
            outs = bass2jax._bass_exec_p.bind(
                *operands,
                out_avals=tuple(out_avals),
                in_names=tuple(all_in_names),
                out_names=tuple(out_names),
                lowering_input_output_aliases=(),
                sim_require_finite=True,
                sim_require_nnan=True,
                nc=nc,
            )
            return tuple(outs)

        devices = jax.devices()[:NCORES]
        self.mesh = Mesh(np.asarray(devices), ("core",))
        n_in = n_params + len(zero_outs)
        self.sharding = jax.sharding.NamedSharding(self.mesh, PartitionSpec("core"))
        self.fn = jax.jit(
            shard_map(
                _body,
                mesh=self.mesh,
                in_specs=(PartitionSpec("core"),) * n_in,
                out_specs=(PartitionSpec("core"),) * len(out_names),
                check_rep=False,
            ),
            keep_unused=True,
        )
        self._dev_args = None

    def stage(self, in_maps):
        """device_put concatenated per-core inputs; cache for reuse."""
        jax = self.jax
        per_core = [[np.asarray(m[n]) for n in self.in_names] for m in in_maps]
        concat_in = [
            np.concatenate([per_core[c][i] for c in range(NCORES)], axis=0)
            for i in range(len(self.in_names))
        ]
        concat_zero = [
            np.zeros((NCORES * z.shape[0], *z.shape[1:]), z.dtype)
            for z in self.zero_outs
        ]
        self._dev_args = [
            jax.device_put(a, self.sharding) for a in concat_in + concat_zero
        ]
        jax.block_until_ready(self._dev_args)

    def execute(self):
        out = self.fn(*self._dev_args)
        self.jax.block_until_ready(out)
        return out

    def run(self, in_maps):
        self.stage(in_maps)
        out_arrs = self.execute()
        return [
            {
                name: np.asarray(out_arrs[i]).reshape(
                    NCORES, *self.out_avals[i].shape
                )[c]
                for i, name in enumerate(self.out_names)
            }
            for c in range(NCORES)
        ]

    def time_execute(self, iters=5):
        import time

        times = []
        for _ in range(iters):
            t0 = time.monotonic()
            self.execute()
            times.append(time.monotonic() - t0)
        return times


_RUNNER = None


def _get_runner():
    global _RUNNER
    if _RUNNER is None:
        _RUNNER = _Runner(_get_nc())
    return _RUNNER


def kernel(query, key, value, Wq, bq, Wk, bk, Wv, bv, Wp, bp):
    global LAST_RESULT
    from concourse import bass_utils

    args = [np.asarray(a) for a in (query, key, value, Wq, bq, Wk, bk, Wv, bv, Wp, bp)]
    query, key, value, Wq, bq, Wk, bk, Wv, bv, Wp, bp = args
    in_maps = _prep_in_maps(query, key, value, Wq, bq, Wk, bk, Wv, bv, Wp, bp)
    res = bass_utils.run_bass_kernel_spmd(
        _get_nc(), in_maps, core_ids=list(range(NCORES))
    )
    LAST_RESULT = res
    parts = [r["out_partial"] for r in res.results]
    gsz = NCORES // B
    out = np.stack(
        [
            np.sum(parts[b * gsz : (b + 1) * gsz], axis=0)
            + bp[None, :].astype(np.float32)
            for b in range(B)
        ]
    )
    return out.astype(np.float32)

